# revision 86
# baseline (speedup 1.0000x reference)
"""Multi-head self-attention (B=2, L=2048, H=16, dh=64) on 8 TRN2 NeuronCores.

Strategy:
  - One SPMD launch; each core owns one head-pair (2 heads, 128 model dims)
    of every batch. Per-batch program sections with lengths padded to 128.
  - Few, large DMAs: X shipped as kc-blocked [128, 8, L] tiles (1 DMA per
    512-col tile), W packed per-projection, key-bias packed once. Each DMA
    instruction costs ~625ns of serialized HWDGE time, so count matters as
    much as bytes. DMA order is hand-scheduled to match compute order.
  - Q projection in fp8e4m3 DoubleRow (2x PE) with residual compensation:
    64*q = X8 @ f8(64*WQ) + (32*(X-X8))8 @ f8(2*WQ); K/V stay bf16.
    All other intermediates bf16 (fp32 PSUM accumulation only).
  - Attention per head: S^T[k, q] scores, exp fused with key-mask bias and
    the 1/512 descale on ScalarE, O^T accumulated with ones-augmented V so
    softmax denominators come out as row 64.
  - No transposes / normalization on device: kernel writes unnormalized
    O^T[65, 2, LQ] (row 64 = denominator); host divides + applies q mask.
  - Overlap: PE warmup matmuls on a zeroed tile cover the DMA-bound head
    (and clear the p-state ramp); attention for q-tile 0 is striped across
    K/V tiles as their DMAs land; scores->exp->AV is software-pipelined several
    chunks deep (depth 5); exp is ~1.2x slower than the matching PE work, so the next
    section's kT/v/qT projections are injected into pure-attention windows
    to keep the local PE/ACT mix balanced.
"""

import math
from contextlib import ExitStack

import ml_dtypes
import numpy as np

import concourse.mybir as mybir
import concourse.tile as tile
from concourse import bacc
from concourse.bass_utils import run_bass_kernel_spmd

F32 = mybir.dt.float32
BF16 = mybir.dt.bfloat16
F8 = mybir.dt.float8e4
DR = mybir.MatmulPerfMode.DoubleRow
EXP = mybir.ActivationFunctionType.Exp
NEG_BIG = 1e12
# Q-path runs in fp8e4m3 with residual compensation: 64*q is accumulated as
# X8 @ f8(64*WQ) + (32*(X-X8))8 @ f8(2*WQ), still half the PE cost of bf16.
# The 1/(64*sqrt(64)) descale folds into the exp's scale argument.
Q_SCALE = 64.0
R_SCALE = 32.0
EXP_SCALE = 1.0 / (Q_SCALE * 8.0)

D_MODEL = 1024
L_FULL = 2048
DH = 64
N_CORES = 8
KC = D_MODEL // 128  # contraction chunks
HW = 128             # one head-pair (2 heads) per core

_nc_cache: dict = {}
TRACE = False


def _tiles(n, w=512):
    return [min(w, n - o) for o in range(0, n, w)]


def _build(cfgs: tuple):
    """cfgs: tuple of (LQ, LK) per batch section."""
    if cfgs in _nc_cache:
        return _nc_cache[cfgs]

    nc = bacc.Bacc("TRN2", target_bir_lowering=False, debug=False,
                   num_devices=N_CORES)

    secs = []
    for i, (LQ, LK) in enumerate(cfgs):
        ktiles = _tiles(LK)
        if i == 0 and ktiles[0] == 512:
            # finer first tiles so the attention stripe starts sooner
            ktiles = [256, 256] + ktiles[1:]
        kt_off, kmap, o = [], [], 0
        for jk, ktw in enumerate(ktiles):
            kt_off.append(o)
            for ck in range(ktw // 128):
                kmap.append((jk, ck))
            o += ktw
        d = dict(LQ=LQ, LK=LK, NKC=LK // 128, NQC=LQ // 128,
                 qtiles=_tiles(LQ), ktiles=ktiles, kmap=kmap, kt_off=kt_off)
        d["xq_d"] = nc.dram_tensor(f"xq{i}", [128, KC, LQ], F8, kind="ExternalInput")
        d["xr_d"] = nc.dram_tensor(f"xr{i}", [128, KC, LQ], F8, kind="ExternalInput")
        d["xk_d"] = nc.dram_tensor(f"xk{i}", [128, KC, LK], BF16, kind="ExternalInput")
        d["xv_d"] = nc.dram_tensor(f"xv{i}", [128, KC, LK], BF16, kind="ExternalInput")
        d["out_d"] = nc.dram_tensor(f"out{i}", [65, 2, LQ], BF16, kind="ExternalOutput")
        secs.append(d)
    NKC_TOT = sum(d["NKC"] for d in secs)
    # merged small inputs: one DMA each for [wq|wql] fp8 and [wk|wv] bf16
    # (each merge saves ~625ns of early HWDGE time); the ones column of
    # v_sb is written by a DVE memset instead of a DMA
    wq8_d = nc.dram_tensor("wq8", [128, KC, 256], F8, kind="ExternalInput")
    wkv_d = nc.dram_tensor("wkv", [128, KC, 256], BF16, kind="ExternalInput")
    kb_d = nc.dram_tensor("kbias", [128, NKC_TOT], F32, kind="ExternalInput")

    with ExitStack() as ctx:
        tc = ctx.enter_context(tile.TileContext(nc))
        const = ctx.enter_context(tc.tile_pool(name="const", bufs=1))
        xpool = ctx.enter_context(tc.tile_pool(name="xp", bufs=1))
        qkp = ctx.enter_context(tc.tile_pool(name="qk", bufs=1))
        vpool = ctx.enter_context(tc.tile_pool(name="vp", bufs=1))
        epool = ctx.enter_context(tc.tile_pool(name="ep", bufs=7))
        fpool = ctx.enter_context(tc.tile_pool(name="fp", bufs=2))
        # PSUM budget (8 banks): 2x2-bank score tiles + 2x1-bank oT
        # accumulators + 2x1-bank projection slots.
        spool = ctx.enter_context(tc.tile_pool(name="ps_s", bufs=2, space="PSUM"))
        bout = ctx.enter_context(tc.tile_pool(name="ps_o", bufs=2, space="PSUM"))
        pjp = ctx.enter_context(tc.tile_pool(name="ps_pj", bufs=2, space="PSUM"))

        # ---- input DMAs (SP queue, no waits: stream back-to-back) ----
        # Order matters: the serial DMA stream gates PE start, so front-load
        # exactly the critical chain for section 0's first attention stripe.
        def xdma(i, d, key, t, halves=False):
            if key in ("xq", "xr"):
                tw, to = d["qtiles"][t], t * 512
            else:
                tw, to = d["ktiles"][t], d["kt_off"][t]
            dt = F8 if key in ("xq", "xr") else BF16
            xt = xpool.tile([128, KC, tw], dt, tag=f"{key}{i}_{t}",
                            name=f"{key}{i}_{t}")
            src = d[key + "_d"]
            if halves == "keys":
                # split by key range: the first v chunk's projection only
                # needs keys 0-127, so it ungates one 128-key piece early
                nc.sync.dma_start(out=xt[:, :, 0:128],
                                  in_=src[:, :, to:to + 128])
                nc.sync.dma_start(out=xt[:, :, 128:tw],
                                  in_=src[:, :, to + 128:to + tw])
            elif halves:
                # two kc-half DMAs so the first accumulation matmuls can
                # start while the second half is still in flight
                nc.sync.dma_start(out=xt[:, 0:KC // 2, :],
                                  in_=src[:, 0:KC // 2, to:to + tw])
                nc.sync.dma_start(out=xt[:, KC // 2:KC, :],
                                  in_=src[:, KC // 2:KC, to:to + tw])
            else:
                nc.sync.dma_start(out=xt, in_=src[:, :, to:to + tw])
            d[key][t] = xt

        for d in secs:
            d["xq"] = [None] * len(d["qtiles"])
            d["xr"] = [None] * len(d["qtiles"])
            d["xk"] = [None] * len(d["ktiles"])
            d["xv"] = [None] * len(d["ktiles"])

        d0 = secs[0]
        w8_t = const.tile([128, KC, 256], F8, name="wq8")
        wkv_t = const.tile([128, KC, 256], BF16, name="wkv")
        # fp8 q/r tiles transfer in ~364ns < the 625ns HWDGE slot, so
        # halving them would pay two full slots — keep them whole; the
        # bf16 k/v tiles (728ns halves) stay split for earlier matmul start
        kb_t = const.tile([128, NKC_TOT], F32, name="kb")
        nc.sync.dma_start(out=w8_t, in_=wq8_d[:, :, :])
        xdma(0, d0, "xq", 0)
        nc.sync.dma_start(out=wkv_t, in_=wkv_d[:, :, :])
        xdma(0, d0, "xk", 0, halves=True)
        nc.sync.dma_start(out=kb_t, in_=kb_d[:, :])
        xdma(0, d0, "xr", 0)
        xdma(0, d0, "xv", 0, halves="keys")
        for t in range(1, len(d0["ktiles"])):
            xdma(0, d0, "xk", t)
            xdma(0, d0, "xv", t)
        for t in range(1, len(d0["qtiles"])):
            xdma(0, d0, "xq", t)
            xdma(0, d0, "xr", t)
        for i, d in enumerate(secs[1:], start=1):
            for t in range(len(d["ktiles"])):
                xdma(i, d, "xk", t)
                xdma(i, d, "xv", t)
            for t in range(len(d["qtiles"])):
                xdma(i, d, "xq", t)
                xdma(i, d, "xr", t)

        kb_off = [0]
        for d in secs[:-1]:
            kb_off.append(kb_off[-1] + d["NKC"])

        # ---- PE warmup: matmuls on a zeroed tile fill the DMA-bound head
        # and clear the p-state ramp before real work arrives ----
        zt = const.tile([128, 512], BF16, name="zwarm")
        nc.vector.memset(zt, 0.0)
        wrm = spool.tile([128, 1024], F32, tag="s", name="warm")
        for n in range(6):
            nc.tensor.matmul(wrm[:, 0:512], lhsT=zt[:, 0:128], rhs=zt,
                             start=True, stop=True)

        # ---- compute ----
        for i, d in enumerate(secs):
            d["qT"] = [None] * len(d["qtiles"])
            d["kT"] = [None] * len(d["ktiles"])
            d["v_sb"] = [None] * d["NKC"]
            d["oT"] = {}
            d["pend"] = []
            d["kb0"] = kb_off[i]

        def proj_q(i, d, t, mid=None):
            """mid: optional emitter run between the main and residual DR
            passes — fills the PE while the xr DMA is still in flight."""
            qw = d["qtiles"][t]
            pj = pjp.tile([128, 512], F32, tag="pj", name=f"pjq{i}_{t}")
            for n, (wlo, xkey) in enumerate(((0, "xq"), (128, "xr"))):
                for c in range(KC // 2):
                    nc.tensor.matmul(
                        pj[:, 0:qw],
                        lhsT=w8_t[:, 2 * c:2 * c + 2, wlo:wlo + 128],
                        rhs=d[xkey][t][:, 2 * c:2 * c + 2, :],
                        start=(n == 0 and c == 0),
                        stop=(n == 1 and c == KC // 2 - 1),
                        perf_mode=DR,
                    )
                if n == 0 and mid is not None:
                    mid()
            qt = qkp.tile([128, qw], BF16, tag=f"qT{i}_{t}", name=f"qT{i}_{t}")
            nc.vector.tensor_copy(qt, pj[:, 0:qw])
            d["qT"][t] = qt

        def build_kv(i, d, jk, k_only=False, v_only=False):
            """Project one 512-key tile of kT and its v chunks."""
            ktw = d["ktiles"][jk]
            if not v_only:
                pj = pjp.tile([128, 512], F32, tag="pj", name=f"pjk{i}_{jk}")
                for kc in range(KC):
                    nc.tensor.matmul(
                        pj[:, 0:ktw],
                        lhsT=wkv_t[:, kc, 0:128],
                        rhs=d["xk"][jk][:, kc, :],
                        start=(kc == 0), stop=(kc == KC - 1),
                    )
                kt = qkp.tile([128, ktw], BF16, tag=f"kT{i}_{jk}",
                              name=f"kT{i}_{jk}")
                nc.vector.tensor_copy(kt, pj[:, 0:ktw])
                d["kT"][jk] = kt
            if k_only:
                return

            kc_base = d["kt_off"][jk] // 128
            for ck in range(ktw // 128):
                kc = kc_base + ck
                pv = pjp.tile([128, 512], F32, tag="pj", name=f"pjv{i}_{kc}")
                for c2 in range(KC):
                    nc.tensor.matmul(
                        pv[:, 0:HW],
                        lhsT=d["xv"][jk][:, c2, ck * 128:(ck + 1) * 128],
                        rhs=wkv_t[:, c2, 128:256],
                        start=(c2 == 0), stop=(c2 == KC - 1),
                    )
                vt = vpool.tile([128, 130], BF16, tag=f"v{i}_{kc}",
                                name=f"v{i}_{kc}")
                v3 = vt.rearrange("p (h c) -> p h c", c=65)
                nc.vector.tensor_copy(
                    v3[:, :, 0:64],
                    pv[:, 0:HW].rearrange("p (h c) -> p h c", c=64))
                nc.vector.memset(v3[:, :, 64:65], 1.0)
                d["v_sb"][kc] = vt

        def emit_scores(i, d, t, kc):
            qw = d["qtiles"][t]
            s = spool.tile([128, 1024], F32, tag="s", name=f"s{i}_{t}_{kc}")
            jk, ck = d["kmap"][kc]
            for h in range(2):
                nc.tensor.matmul(
                    s[:, h * 512:h * 512 + qw],
                    lhsT=d["kT"][jk][h * 64:(h + 1) * 64, ck * 128:(ck + 1) * 128],
                    rhs=d["qT"][t][h * 64:(h + 1) * 64, :],
                    start=True, stop=True,
                    tile_position=(h * 64, 0),
                )
            e = epool.tile([128, 1024], BF16, tag="e", name=f"e{i}_{t}_{kc}")
            s_view = s.rearrange("p (b c) -> p b c", c=512)[:, :, 0:qw]
            nc.scalar.activation(
                e.rearrange("p (b c) -> p b c", c=512)[:, :, 0:qw],
                s_view, EXP, bias=kb_t[:, d["kb0"] + kc:d["kb0"] + kc + 1],
                scale=EXP_SCALE)
            d["pend"].append((t, kc, e, qw))

        def emit_av(i, d):
            t, kc, e, qw = d["pend"].pop(0)
            if kc == 0:
                for h in range(2):
                    d["oT"][(t, h)] = bout.tile([65, 512], F32, tag="oT",
                                                name=f"oT{i}_{t}_{h}")
            for h in range(2):
                nc.tensor.matmul(
                    d["oT"][(t, h)][:, 0:qw],
                    lhsT=d["v_sb"][kc][:, 65 * h:65 * h + 65],
                    rhs=e[:, h * 512:h * 512 + qw],
                    start=(kc == 0), stop=(kc == d["NKC"] - 1),
                )
            if kc == d["NKC"] - 1:
                emit_finalize(i, d, t)

        def emit_finalize(i, d, t):
            qw = d["qtiles"][t]
            ft = fpool.tile([65, 1024], BF16, tag="of", name=f"of{i}_{t}")
            f3 = ft.rearrange("p (h c) -> p h c", c=512)
            for h in range(2):
                nc.vector.tensor_copy(f3[:, h, 0:qw], d["oT"][(t, h)][:, 0:qw])
            nc.sync.dma_start(
                out=d["out_d"][:, :, t * 512:t * 512 + qw],
                in_=f3[:, :, 0:qw])

        # Global schedule. Attention chunks are ACT-bound (exp ~1040ns vs
        # ~850ns of PE per chunk), so pure-attention windows stall the PE
        # pipeline on exp latency. Counter it by injecting the NEXT
        # section's kT/v projection tiles into this section's post-stripe
        # attention windows, keeping the local PE/ACT mix balanced.
        for i, d in enumerate(secs):
            NKC, NQT = d["NKC"], len(d["qtiles"])
            nxt = secs[i + 1] if i + 1 < len(secs) else None
            # work to spread into this section's attention: the next
            # section's kT/v tiles and its first qT projection
            nxt_tasks = []
            if nxt:
                nxt_tasks = [("kv", jk) for jk in range(len(nxt["ktiles"]))]
                nxt_tasks.append(("q0",))

            def run_task(task):
                if task[0] == "kv":
                    build_kv(i + 1, nxt, task[1])
                else:
                    proj_q(i + 1, nxt, 0)

            if d["qT"][0] is None:
                if i == 0:
                    # kT of the first key tile rides between the Q main and
                    # residual passes (xr lands after xk in the DMA stream)
                    proj_q(i, d, 0,
                           mid=lambda: build_kv(i, d, 0, k_only=True))
                    build_kv(i, d, 0, v_only=True)
                else:
                    proj_q(i, d, 0)
            # stripe: build kT/v per key tile (unless prebuilt), then run
            # q-tile 0's attention over that tile's chunks
            last_jk = len(d["ktiles"]) - 1
            for jk, ktw in enumerate(d["ktiles"]):
                if d["kT"][jk] is None:
                    build_kv(i, d, jk)
                kc_base = d["kt_off"][jk] // 128
                for ck in range(ktw // 128):
                    emit_scores(i, d, 0, kc_base + ck)
                    if len(d["pend"]) > 5:
                        emit_av(i, d)
                    if jk == last_jk and ck == 0 and NQT > 1:
                        proj_q(i, d, 1)

            # remaining q-tiles, with next-section tasks spread in;
            # q-tile t+1 is projected from the middle of tile t's loop
            n_steps = (NQT - 1) * NKC
            inject_at = {}
            if nxt_tasks and n_steps > 0:
                stride = max(1, n_steps // (len(nxt_tasks) + 1))
                for n, task in enumerate(nxt_tasks):
                    inject_at[min(n_steps - 1 - n, (n + 1) * stride)] = task
            step = 0
            for t in range(1, NQT):
                if d["qT"][t] is None:
                    proj_q(i, d, t)
                for kc in range(NKC):
                    emit_scores(i, d, t, kc)
                    if len(d["pend"]) > 5:
                        emit_av(i, d)
                    if kc == NKC // 2 and t + 1 < NQT:
                        proj_q(i, d, t + 1)
                    if step in inject_at:
                        run_task(inject_at[step])
                    step += 1
            if NQT == 1:
                for task in nxt_tasks:
                    run_task(task)
            while d["pend"]:
                emit_av(i, d)

    nc.compile()
    _nc_cache[cfgs] = nc
    return nc


def _pad128(n: int) -> int:
    return min(L_FULL, max(128, int(math.ceil(n / 128)) * 128))


def _kc_block(x_t: np.ndarray, dt=ml_dtypes.bfloat16) -> np.ndarray:
    """[1024, L] -> [128, KC, L] kc-blocked."""
    L = x_t.shape[1]
    return np.ascontiguousarray(
        x_t.reshape(KC, 128, L).transpose(1, 0, 2)).astype(dt)


def kernel(Q_seq, K_seq, V_seq, q_len, v_len, WQ, WK, WV):
    Q_seq = np.asarray(Q_seq, dtype=np.float32)
    K_seq = np.asarray(K_seq, dtype=np.float32)
    V_seq = np.asarray(V_seq, dtype=np.float32)
    WQ = np.asarray(WQ, dtype=np.float32)
    WK = np.asarray(WK, dtype=np.float32)
    WV = np.asarray(WV, dtype=np.float32)
    ql = np.asarray(q_len).ravel().astype(np.int64)
    vl = np.asarray(v_len).ravel().astype(np.int64)
    B = Q_seq.shape[0]

    WQs = WQ * np.float32(Q_SCALE)
    cfgs = tuple((_pad128(int(ql[b])), _pad128(int(vl[b]))) for b in range(B))
    nc = _build(cfgs)

    kb_parts = []
    for b in range(B):
        LK = cfgs[b][1]
        kbias = np.where(np.arange(LK) < vl[b], 0.0, -NEG_BIG).astype(np.float32)
        kb_parts.append(kbias.reshape(LK // 128, 128).T)
    kb_all = np.ascontiguousarray(np.concatenate(kb_parts, axis=1))

    in_maps = [dict() for _ in range(N_CORES)]
    xs = {}
    for b in range(B):
        LQ, LK = cfgs[b]
        xq_t = np.ascontiguousarray(Q_seq[b, :LQ, :].T)
        xq8 = xq_t.astype(ml_dtypes.float8_e4m3)
        xr8 = ((xq_t - xq8.astype(np.float32)) * np.float32(R_SCALE))
        xs[f"xq{b}"] = _kc_block(xq8.astype(np.float32), ml_dtypes.float8_e4m3)
        xs[f"xr{b}"] = _kc_block(xr8, ml_dtypes.float8_e4m3)
        xs[f"xk{b}"] = _kc_block(K_seq[b, :LK, :].T)
        xs[f"xv{b}"] = _kc_block(V_seq[b, :LK, :].T)
    WQl = WQ * np.float32(Q_SCALE / R_SCALE)
    for g in range(N_CORES):
        sl = slice(g * HW, (g + 1) * HW)

        def wpack(Wa, Wb, wdt):
            wp = np.concatenate(
                [Wa[:, sl].reshape(KC, 128, 128).transpose(1, 0, 2),
                 Wb[:, sl].reshape(KC, 128, 128).transpose(1, 0, 2)], axis=2)
            return np.ascontiguousarray(wp).astype(wdt)

        in_maps[g]["wq8"] = wpack(WQs, WQl, ml_dtypes.float8_e4m3)
        in_maps[g]["wkv"] = wpack(WK, WV, ml_dtypes.bfloat16)
        in_maps[g]["kbias"] = kb_all
        in_maps[g].update(xs)

    res = run_bass_kernel_spmd(nc, in_maps, list(range(N_CORES)), trace=TRACE)
    kernel.last_results = [res]
    kernel.last_exec_ns = res.exec_time_ns or 0

    O = np.zeros((B, L_FULL, D_MODEL), dtype=np.float32)
    for b in range(B):
        LQ = cfgs[b][0]
        n_valid = int(ql[b])
        for g in range(N_CORES):
            out = np.asarray(res.results[g][f"out{b}"], dtype=np.float32)
            for h in range(2):
                oh = out[0:64, h, :] / out[64:65, h, :]
                O[b, :LQ, g * HW + h * 64:g * HW + (h + 1) * 64] = oh.T
        O[b, n_valid:, :] = 0.0
    return O


# revision 87
# speedup vs baseline: 1.0219x; 1.0219x over previous
"""Multi-head self-attention (B=2, L=2048, H=16, dh=64) on 8 TRN2 NeuronCores.

Strategy:
  - One SPMD launch; each core owns one head-pair (2 heads, 128 model dims)
    of every batch. Per-batch program sections with lengths padded to 128.
  - Few, large DMAs: X shipped as kc-blocked [128, 8, L] tiles (1 DMA per
    512-col tile), W packed per-projection, key-bias packed once. Each DMA
    instruction costs ~625ns of serialized HWDGE time, so count matters as
    much as bytes. DMA order is hand-scheduled to match compute order.
  - Q projection in fp8e4m3 DoubleRow (2x PE) with residual compensation:
    64*q = X8 @ f8(64*WQ) + (32*(X-X8))8 @ f8(2*WQ); K/V stay bf16.
    All other intermediates bf16 (fp32 PSUM accumulation only).
  - Attention per head: S^T[k, q] scores, exp fused with key-mask bias and
    the 1/512 descale on ScalarE, O^T accumulated with ones-augmented V so
    softmax denominators come out as row 64.
  - No transposes / normalization on device: kernel writes unnormalized
    O^T[65, 2, LQ] (row 64 = denominator); host divides + applies q mask.
  - Overlap: PE warmup matmuls on a zeroed tile cover the DMA-bound head
    (and clear the p-state ramp); attention for q-tile 0 is striped across
    K/V tiles as their DMAs land; scores->exp->AV is software-pipelined several
    chunks deep (depth 5); exp is ~1.2x slower than the matching PE work, so the next
    section's kT/v/qT projections are injected into pure-attention windows
    to keep the local PE/ACT mix balanced.
"""

import math
from contextlib import ExitStack

import ml_dtypes
import numpy as np

import concourse.mybir as mybir
import concourse.tile as tile
from concourse import bacc
from concourse.bass_utils import run_bass_kernel_spmd

F32 = mybir.dt.float32
BF16 = mybir.dt.bfloat16
F8 = mybir.dt.float8e4
DR = mybir.MatmulPerfMode.DoubleRow
EXP = mybir.ActivationFunctionType.Exp
NEG_BIG = 1e12
# Q-path runs in fp8e4m3 with residual compensation: 64*q is accumulated as
# X8 @ f8(64*WQ) + (32*(X-X8))8 @ f8(2*WQ), still half the PE cost of bf16.
# The 1/(64*sqrt(64)) descale folds into the exp's scale argument.
Q_SCALE = 64.0
R_SCALE = 32.0
EXP_SCALE = 1.0 / (Q_SCALE * 8.0)

D_MODEL = 1024
L_FULL = 2048
DH = 64
N_CORES = 8
KC = D_MODEL // 128  # contraction chunks
HW = 128             # one head-pair (2 heads) per core

_nc_cache: dict = {}
TRACE = False


def _tiles(n, w=512):
    return [min(w, n - o) for o in range(0, n, w)]


def _build(cfgs: tuple):
    """cfgs: tuple of (LQ, LK) per batch section."""
    if cfgs in _nc_cache:
        return _nc_cache[cfgs]

    nc = bacc.Bacc("TRN2", target_bir_lowering=False, debug=False,
                   num_devices=N_CORES)

    secs = []
    for i, (LQ, LK) in enumerate(cfgs):
        ktiles = _tiles(LK)
        if i == 0 and ktiles[0] == 512:
            # finer first tiles so the attention stripe starts sooner
            ktiles = [256, 256] + ktiles[1:]
        kt_off, kmap, o = [], [], 0
        for jk, ktw in enumerate(ktiles):
            kt_off.append(o)
            for ck in range(ktw // 128):
                kmap.append((jk, ck))
            o += ktw
        d = dict(LQ=LQ, LK=LK, NKC=LK // 128, NQC=LQ // 128,
                 qtiles=_tiles(LQ), ktiles=ktiles, kmap=kmap, kt_off=kt_off)
        d["xq_d"] = nc.dram_tensor(f"xq{i}", [128, KC, LQ], F8, kind="ExternalInput")
        d["xr_d"] = nc.dram_tensor(f"xr{i}", [128, KC, LQ], F8, kind="ExternalInput")
        d["xk_d"] = nc.dram_tensor(f"xk{i}", [128, KC, LK], BF16, kind="ExternalInput")
        d["xv_d"] = nc.dram_tensor(f"xv{i}", [128, KC, LK], BF16, kind="ExternalInput")
        d["out_d"] = nc.dram_tensor(f"out{i}", [65, 2, LQ], BF16, kind="ExternalOutput")
        secs.append(d)
    NKC_TOT = sum(d["NKC"] for d in secs)
    # merged small inputs: one DMA each for [wq|wql] fp8 and [wk|wv] bf16
    # (each merge saves ~625ns of early HWDGE time); the ones column of
    # v_sb is written by a DVE memset instead of a DMA
    wq8_d = nc.dram_tensor("wq8", [128, KC, 256], F8, kind="ExternalInput")
    wkv_d = nc.dram_tensor("wkv", [128, KC, 256], BF16, kind="ExternalInput")
    kb_d = nc.dram_tensor("kbias", [128, NKC_TOT], F32, kind="ExternalInput")

    with ExitStack() as ctx:
        tc = ctx.enter_context(tile.TileContext(nc))
        const = ctx.enter_context(tc.tile_pool(name="const", bufs=1))
        xpool = ctx.enter_context(tc.tile_pool(name="xp", bufs=1))
        qkp = ctx.enter_context(tc.tile_pool(name="qk", bufs=1))
        vpool = ctx.enter_context(tc.tile_pool(name="vp", bufs=1))
        epool = ctx.enter_context(tc.tile_pool(name="ep", bufs=7))
        fpool = ctx.enter_context(tc.tile_pool(name="fp", bufs=2))
        # PSUM budget (8 banks): 2x2-bank score tiles + 2x1-bank oT
        # accumulators + 2x1-bank projection slots.
        spool = ctx.enter_context(tc.tile_pool(name="ps_s", bufs=2, space="PSUM"))
        bout = ctx.enter_context(tc.tile_pool(name="ps_o", bufs=2, space="PSUM"))
        pjp = ctx.enter_context(tc.tile_pool(name="ps_pj", bufs=2, space="PSUM"))

        # ---- input DMAs (SP queue, no waits: stream back-to-back) ----
        # Order matters: the serial DMA stream gates PE start, so front-load
        # exactly the critical chain for section 0's first attention stripe.
        def xdma(i, d, key, t, halves=False):
            if key in ("xq", "xr"):
                tw, to = d["qtiles"][t], t * 512
            else:
                tw, to = d["ktiles"][t], d["kt_off"][t]
            dt = F8 if key in ("xq", "xr") else BF16
            xt = xpool.tile([128, KC, tw], dt, tag=f"{key}{i}_{t}",
                            name=f"{key}{i}_{t}")
            src = d[key + "_d"]
            if halves == "keys":
                # split by key range: the first v chunk's projection only
                # needs keys 0-127, so it ungates one 128-key piece early
                nc.sync.dma_start(out=xt[:, :, 0:128],
                                  in_=src[:, :, to:to + 128])
                nc.sync.dma_start(out=xt[:, :, 128:tw],
                                  in_=src[:, :, to + 128:to + tw])
            elif halves:
                # two kc-half DMAs so the first accumulation matmuls can
                # start while the second half is still in flight
                nc.sync.dma_start(out=xt[:, 0:KC // 2, :],
                                  in_=src[:, 0:KC // 2, to:to + tw])
                nc.sync.dma_start(out=xt[:, KC // 2:KC, :],
                                  in_=src[:, KC // 2:KC, to:to + tw])
            else:
                nc.sync.dma_start(out=xt, in_=src[:, :, to:to + tw])
            d[key][t] = xt

        for d in secs:
            d["xq"] = [None] * len(d["qtiles"])
            d["xr"] = [None] * len(d["qtiles"])
            d["xk"] = [None] * len(d["ktiles"])
            d["xv"] = [None] * len(d["ktiles"])

        d0 = secs[0]
        w8_t = const.tile([128, KC, 256], F8, name="wq8")
        wkv_t = const.tile([128, KC, 256], BF16, name="wkv")
        # fp8 q/r tiles transfer in ~364ns < the 625ns HWDGE slot, so
        # halving them would pay two full slots — keep them whole; the
        # bf16 k/v tiles (728ns halves) stay split for earlier matmul start
        kb_t = const.tile([128, NKC_TOT], F32, name="kb")
        nc.sync.dma_start(out=w8_t, in_=wq8_d[:, :, :])
        xdma(0, d0, "xq", 0)
        nc.sync.dma_start(out=wkv_t, in_=wkv_d[:, :, :])
        xdma(0, d0, "xk", 0, halves=True)
        nc.sync.dma_start(out=kb_t, in_=kb_d[:, :])
        xdma(0, d0, "xr", 0)
        xdma(0, d0, "xv", 0, halves=True)
        for t in range(1, len(d0["ktiles"])):
            xdma(0, d0, "xk", t)
            xdma(0, d0, "xv", t)
        for t in range(1, len(d0["qtiles"])):
            xdma(0, d0, "xq", t)
            xdma(0, d0, "xr", t)
        for i, d in enumerate(secs[1:], start=1):
            for t in range(len(d["ktiles"])):
                xdma(i, d, "xk", t)
                xdma(i, d, "xv", t)
            for t in range(len(d["qtiles"])):
                xdma(i, d, "xq", t)
                xdma(i, d, "xr", t)

        kb_off = [0]
        for d in secs[:-1]:
            kb_off.append(kb_off[-1] + d["NKC"])

        # ---- PE warmup: matmuls on a zeroed tile fill the DMA-bound head
        # and clear the p-state ramp before real work arrives ----
        zt = const.tile([128, 512], BF16, name="zwarm")
        nc.vector.memset(zt, 0.0)
        wrm = spool.tile([128, 1024], F32, tag="s", name="warm")
        for n in range(6):
            nc.tensor.matmul(wrm[:, 0:512], lhsT=zt[:, 0:128], rhs=zt,
                             start=True, stop=True)

        # ---- compute ----
        for i, d in enumerate(secs):
            d["qT"] = [None] * len(d["qtiles"])
            d["kT"] = [None] * len(d["ktiles"])
            d["v_sb"] = [None] * d["NKC"]
            d["oT"] = {}
            d["pend"] = []
            d["kb0"] = kb_off[i]

        def proj_q(i, d, t, mid=None):
            """mid: optional emitter run between the main and residual DR
            passes — fills the PE while the xr DMA is still in flight."""
            qw = d["qtiles"][t]
            pj = pjp.tile([128, 512], F32, tag="pj", name=f"pjq{i}_{t}")
            for n, (wlo, xkey) in enumerate(((0, "xq"), (128, "xr"))):
                for c in range(KC // 2):
                    nc.tensor.matmul(
                        pj[:, 0:qw],
                        lhsT=w8_t[:, 2 * c:2 * c + 2, wlo:wlo + 128],
                        rhs=d[xkey][t][:, 2 * c:2 * c + 2, :],
                        start=(n == 0 and c == 0),
                        stop=(n == 1 and c == KC // 2 - 1),
                        perf_mode=DR,
                    )
                if n == 0 and mid is not None:
                    mid()
            qt = qkp.tile([128, qw], BF16, tag=f"qT{i}_{t}", name=f"qT{i}_{t}")
            nc.vector.tensor_copy(qt, pj[:, 0:qw])
            d["qT"][t] = qt

        def build_kv(i, d, jk, k_only=False, v_only=False):
            """Project one 512-key tile of kT and its v chunks."""
            ktw = d["ktiles"][jk]
            if not v_only:
                pj = pjp.tile([128, 512], F32, tag="pj", name=f"pjk{i}_{jk}")
                for kc in range(KC):
                    nc.tensor.matmul(
                        pj[:, 0:ktw],
                        lhsT=wkv_t[:, kc, 0:128],
                        rhs=d["xk"][jk][:, kc, :],
                        start=(kc == 0), stop=(kc == KC - 1),
                    )
                kt = qkp.tile([128, ktw], BF16, tag=f"kT{i}_{jk}",
                              name=f"kT{i}_{jk}")
                nc.vector.tensor_copy(kt, pj[:, 0:ktw])
                d["kT"][jk] = kt
            if k_only:
                return

            kc_base = d["kt_off"][jk] // 128
            for ck in range(ktw // 128):
                kc = kc_base + ck
                pv = pjp.tile([128, 512], F32, tag="pj", name=f"pjv{i}_{kc}")
                for c2 in range(KC):
                    nc.tensor.matmul(
                        pv[:, 0:HW],
                        lhsT=d["xv"][jk][:, c2, ck * 128:(ck + 1) * 128],
                        rhs=wkv_t[:, c2, 128:256],
                        start=(c2 == 0), stop=(c2 == KC - 1),
                    )
                vt = vpool.tile([128, 130], BF16, tag=f"v{i}_{kc}",
                                name=f"v{i}_{kc}")
                v3 = vt.rearrange("p (h c) -> p h c", c=65)
                nc.vector.tensor_copy(
                    v3[:, :, 0:64],
                    pv[:, 0:HW].rearrange("p (h c) -> p h c", c=64))
                nc.vector.memset(v3[:, :, 64:65], 1.0)
                d["v_sb"][kc] = vt

        def emit_scores(i, d, t, kc):
            qw = d["qtiles"][t]
            s = spool.tile([128, 1024], F32, tag="s", name=f"s{i}_{t}_{kc}")
            jk, ck = d["kmap"][kc]
            for h in range(2):
                nc.tensor.matmul(
                    s[:, h * 512:h * 512 + qw],
                    lhsT=d["kT"][jk][h * 64:(h + 1) * 64, ck * 128:(ck + 1) * 128],
                    rhs=d["qT"][t][h * 64:(h + 1) * 64, :],
                    start=True, stop=True,
                    tile_position=(h * 64, 0),
                )
            e = epool.tile([128, 1024], BF16, tag="e", name=f"e{i}_{t}_{kc}")
            s_view = s.rearrange("p (b c) -> p b c", c=512)[:, :, 0:qw]
            nc.scalar.activation(
                e.rearrange("p (b c) -> p b c", c=512)[:, :, 0:qw],
                s_view, EXP, bias=kb_t[:, d["kb0"] + kc:d["kb0"] + kc + 1],
                scale=EXP_SCALE)
            d["pend"].append((t, kc, e, qw))

        def emit_av(i, d):
            t, kc, e, qw = d["pend"].pop(0)
            if kc == 0:
                for h in range(2):
                    d["oT"][(t, h)] = bout.tile([65, 512], F32, tag="oT",
                                                name=f"oT{i}_{t}_{h}")
            for h in range(2):
                nc.tensor.matmul(
                    d["oT"][(t, h)][:, 0:qw],
                    lhsT=d["v_sb"][kc][:, 65 * h:65 * h + 65],
                    rhs=e[:, h * 512:h * 512 + qw],
                    start=(kc == 0), stop=(kc == d["NKC"] - 1),
                )
            if kc == d["NKC"] - 1:
                emit_finalize(i, d, t)

        def emit_finalize(i, d, t):
            qw = d["qtiles"][t]
            ft = fpool.tile([65, 1024], BF16, tag="of", name=f"of{i}_{t}")
            f3 = ft.rearrange("p (h c) -> p h c", c=512)
            for h in range(2):
                nc.vector.tensor_copy(f3[:, h, 0:qw], d["oT"][(t, h)][:, 0:qw])
            nc.sync.dma_start(
                out=d["out_d"][:, :, t * 512:t * 512 + qw],
                in_=f3[:, :, 0:qw])

        # Global schedule. Attention chunks are ACT-bound (exp ~1040ns vs
        # ~850ns of PE per chunk), so pure-attention windows stall the PE
        # pipeline on exp latency. Counter it by injecting the NEXT
        # section's kT/v projection tiles into this section's post-stripe
        # attention windows, keeping the local PE/ACT mix balanced.
        for i, d in enumerate(secs):
            NKC, NQT = d["NKC"], len(d["qtiles"])
            nxt = secs[i + 1] if i + 1 < len(secs) else None
            # work to spread into this section's attention: the next
            # section's kT/v tiles and its first qT projection
            nxt_tasks = []
            if nxt:
                nxt_tasks = [("kv", jk) for jk in range(len(nxt["ktiles"]))]
                nxt_tasks.append(("q0",))

            def run_task(task):
                if task[0] == "kv":
                    build_kv(i + 1, nxt, task[1])
                else:
                    proj_q(i + 1, nxt, 0)

            if d["qT"][0] is None:
                if i == 0:
                    # kT of the first key tile rides between the Q main and
                    # residual passes (xr lands after xk in the DMA stream)
                    proj_q(i, d, 0,
                           mid=lambda: build_kv(i, d, 0, k_only=True))
                    build_kv(i, d, 0, v_only=True)
                else:
                    proj_q(i, d, 0)
            # stripe: build kT/v per key tile (unless prebuilt), then run
            # q-tile 0's attention over that tile's chunks
            last_jk = len(d["ktiles"]) - 1
            for jk, ktw in enumerate(d["ktiles"]):
                if d["kT"][jk] is None:
                    build_kv(i, d, jk)
                kc_base = d["kt_off"][jk] // 128
                for ck in range(ktw // 128):
                    emit_scores(i, d, 0, kc_base + ck)
                    if len(d["pend"]) > 5:
                        emit_av(i, d)
                    if jk == last_jk and ck == 0 and NQT > 1:
                        proj_q(i, d, 1)

            # remaining q-tiles, with next-section tasks spread in;
            # q-tile t+1 is projected from the middle of tile t's loop
            n_steps = (NQT - 1) * NKC
            inject_at = {}
            if nxt_tasks and n_steps > 0:
                stride = max(1, n_steps // (len(nxt_tasks) + 1))
                for n, task in enumerate(nxt_tasks):
                    inject_at[min(n_steps - 1 - n, (n + 1) * stride)] = task
            step = 0
            for t in range(1, NQT):
                if d["qT"][t] is None:
                    proj_q(i, d, t)
                for kc in range(NKC):
                    emit_scores(i, d, t, kc)
                    if len(d["pend"]) > 5:
                        emit_av(i, d)
                    if kc == NKC // 2 and t + 1 < NQT:
                        proj_q(i, d, t + 1)
                    if step in inject_at:
                        run_task(inject_at[step])
                    step += 1
            if NQT == 1:
                for task in nxt_tasks:
                    run_task(task)
            while d["pend"]:
                emit_av(i, d)

    nc.compile()
    _nc_cache[cfgs] = nc
    return nc


def _pad128(n: int) -> int:
    return min(L_FULL, max(128, int(math.ceil(n / 128)) * 128))


def _kc_block(x_t: np.ndarray, dt=ml_dtypes.bfloat16) -> np.ndarray:
    """[1024, L] -> [128, KC, L] kc-blocked."""
    L = x_t.shape[1]
    return np.ascontiguousarray(
        x_t.reshape(KC, 128, L).transpose(1, 0, 2)).astype(dt)


def kernel(Q_seq, K_seq, V_seq, q_len, v_len, WQ, WK, WV):
    Q_seq = np.asarray(Q_seq, dtype=np.float32)
    K_seq = np.asarray(K_seq, dtype=np.float32)
    V_seq = np.asarray(V_seq, dtype=np.float32)
    WQ = np.asarray(WQ, dtype=np.float32)
    WK = np.asarray(WK, dtype=np.float32)
    WV = np.asarray(WV, dtype=np.float32)
    ql = np.asarray(q_len).ravel().astype(np.int64)
    vl = np.asarray(v_len).ravel().astype(np.int64)
    B = Q_seq.shape[0]

    WQs = WQ * np.float32(Q_SCALE)
    cfgs = tuple((_pad128(int(ql[b])), _pad128(int(vl[b]))) for b in range(B))
    nc = _build(cfgs)

    kb_parts = []
    for b in range(B):
        LK = cfgs[b][1]
        kbias = np.where(np.arange(LK) < vl[b], 0.0, -NEG_BIG).astype(np.float32)
        kb_parts.append(kbias.reshape(LK // 128, 128).T)
    kb_all = np.ascontiguousarray(np.concatenate(kb_parts, axis=1))

    in_maps = [dict() for _ in range(N_CORES)]
    xs = {}
    for b in range(B):
        LQ, LK = cfgs[b]
        xq_t = np.ascontiguousarray(Q_seq[b, :LQ, :].T)
        xq8 = xq_t.astype(ml_dtypes.float8_e4m3)
        xr8 = ((xq_t - xq8.astype(np.float32)) * np.float32(R_SCALE))
        xs[f"xq{b}"] = _kc_block(xq8.astype(np.float32), ml_dtypes.float8_e4m3)
        xs[f"xr{b}"] = _kc_block(xr8, ml_dtypes.float8_e4m3)
        xs[f"xk{b}"] = _kc_block(K_seq[b, :LK, :].T)
        xs[f"xv{b}"] = _kc_block(V_seq[b, :LK, :].T)
    WQl = WQ * np.float32(Q_SCALE / R_SCALE)
    for g in range(N_CORES):
        sl = slice(g * HW, (g + 1) * HW)

        def wpack(Wa, Wb, wdt):
            wp = np.concatenate(
                [Wa[:, sl].reshape(KC, 128, 128).transpose(1, 0, 2),
                 Wb[:, sl].reshape(KC, 128, 128).transpose(1, 0, 2)], axis=2)
            return np.ascontiguousarray(wp).astype(wdt)

        in_maps[g]["wq8"] = wpack(WQs, WQl, ml_dtypes.float8_e4m3)
        in_maps[g]["wkv"] = wpack(WK, WV, ml_dtypes.bfloat16)
        in_maps[g]["kbias"] = kb_all
        in_maps[g].update(xs)

    res = run_bass_kernel_spmd(nc, in_maps, list(range(N_CORES)), trace=TRACE)
    kernel.last_results = [res]
    kernel.last_exec_ns = res.exec_time_ns or 0

    O = np.zeros((B, L_FULL, D_MODEL), dtype=np.float32)
    for b in range(B):
        LQ = cfgs[b][0]
        n_valid = int(ql[b])
        for g in range(N_CORES):
            out = np.asarray(res.results[g][f"out{b}"], dtype=np.float32)
            for h in range(2):
                oh = out[0:64, h, :] / out[64:65, h, :]
                O[b, :LQ, g * HW + h * 64:g * HW + (h + 1) * 64] = oh.T
        O[b, n_valid:, :] = 0.0
    return O
